# revision 121
# baseline (speedup 1.0000x reference)
# Trainium2 Bass kernel for a causal multi-head attention block.
#
# Reference computation (fp32):
#   qkv = x @ w_attn + b_attn ; split into q,k,v heads (N=16, H=64)
#   scores = q @ k^T / sqrt(H), causal mask, softmax over keys
#   out = (weights @ v) reshaped, then out @ w_proj + b_proj
#
# Sharding: 8 cores = 2 batches x 4 head-groups (4 heads each).
#   - batch data-parallel, heads tensor-parallel (c_attn columns / c_proj rows)
#   - each core emits a partial [T, D] projection output; host sums the 4
#     head-group partials per batch and adds b_proj (the gather step).
#
# Device-side layout / schedule (tuned against the TimelineSim cost model,
# where matmul cost = output-free-size x cycles-per-row and Ldweights is
# free, so PE time ~ sum of output columns x k-steps):
#   - x arrives PRE-TRANSPOSED (x^T, bf16) from the host, so the qkv
#     projections need no on-device transposes at all.
#   - scores are computed TRANSPOSED (S^T[s,t]) so exp(S^T) tiles feed the
#     weights@V matmul directly (contraction over s = partition dim); row
#     sums come free via a ones-column in V.
#   - all PE operands are bf16 (1 cycle/row at any width -> causal score
#     matmuls are trimmed to the valid t-range at 128-col granularity).
#   - attention is software-pipelined: AV(i) is emitted after scores(i+1),
#     and the final AV of each head-pair is deferred into the next emission
#     section, so the in-order PE stream never waits on a fresh exp.
#   - the final block masks its causal diagonal IN PSUM (extra -200-additive
#     matmul) instead of the DVE mask-multiply, shortening the endgame
#     dependency chains.
#   - projection of t-blocks 0..2 is interleaved into the final attention
#     block's emission (one unit per full-width s-tile): that block is
#     ACT(exp)-bound, so the PE fills its dependency stalls with ready
#     projection work.  y goes out in bf16 (host sums partials in fp32).
#   - startup: k=0,1 land as single-chunk DMAs, the rest pair-granular, the
#     V weights / later t-quarters / consts as few large or SWDGE DMAs
#     (HWDGE serializes ~628ns per DMA instruction).

import math

import numpy as np

B, T, D = 2, 2048, 1024
NHEAD, H = 16, 64
HPC = 4            # heads per core
CD = HPC * H       # 256 head-dim columns per core
N_CORES = 8
P = 128            # partitions
TT = T // P        # 16 t-tiles of 128
TB = T // 512      # 4 t-blocks of 512
KD = D // P        # 8 contraction tiles over D

_CACHE = {}


def _build_module(mm_dt_name: str):
    import contextlib

    import concourse.bass as bass  # noqa: F401
    import concourse.mybir as mybir
    import concourse.tile as tile
    from concourse import bacc

    f32 = mybir.dt.float32
    bf16 = mybir.dt.bfloat16

    nc = bacc.Bacc("TRN2", target_bir_lowering=False, debug=False)

    xt_d = nc.dram_tensor("xt", [D, T], bf16, kind="ExternalInput").ap()
    wqkv_d = nc.dram_tensor("wqkv", [D, 3 * CD], bf16, kind="ExternalInput").ap()
    bqk_d = nc.dram_tensor("bqk", [P, 4], f32, kind="ExternalInput").ap()
    bv_d = nc.dram_tensor("bv", [P, CD], bf16, kind="ExternalInput").ap()
    wp_d = nc.dram_tensor("wp", [CD, D], bf16, kind="ExternalInput").ap()
    ident_d = nc.dram_tensor("ident", [P, P], bf16, kind="ExternalInput").ap()
    mask_d = nc.dram_tensor("mask", [P, P], bf16, kind="ExternalInput").ap()
    maskn_d = nc.dram_tensor("maskn", [P, P], bf16, kind="ExternalInput").ap()
    onescol_d = nc.dram_tensor("onescol", [P, 2 * HPC], bf16, kind="ExternalInput").ap()
    y_d = nc.dram_tensor("y", [T, D], bf16, kind="ExternalOutput").ap()

    with tile.TileContext(nc) as tc, contextlib.ExitStack() as ctx:
        const_p = ctx.enter_context(tc.tile_pool(name="const", bufs=1))
        w_p = ctx.enter_context(tc.tile_pool(name="weights", bufs=1))
        x_p = ctx.enter_context(tc.tile_pool(name="xin", bufs=1))
        qkt_p = ctx.enter_context(tc.tile_pool(name="qkt", bufs=1))
        v_p = ctx.enter_context(tc.tile_pool(name="vbuf", bufs=1))
        e_p = ctx.enter_context(tc.tile_pool(name="epool", bufs=16))
        attn_p = ctx.enter_context(tc.tile_pool(name="attn", bufs=1))
        small_p = ctx.enter_context(tc.tile_pool(name="small", bufs=8))
        # single PSUM pool, 8 banks total:
        #   wps   [128,512]x2  (qk^T/V psums + proj)                  2 banks
        #   sp    [128,1024]x2 (scores)                               4 banks
        #   accp* [128,264]x2  (AV accumulators, 4 groups per bank)   2 banks
        psp = ctx.enter_context(tc.tile_pool(name="psp", bufs=2, space="PSUM"))

        # ---- loads: the phase-1-critical (wqkv, x^T) pairs go FIRST as few
        # large HWDGE transfers (HWDGE serializes ~628ns per DMA); consts and
        # wp are only needed later and go via SWDGE (gpsimd) off that path ----
        # qk-columns of wqkv (critical for the first block) and the first
        # t-quarter of x^T land first; v-columns and later t-quarters follow.
        # k=0,1 go as single-chunk tiles so the very first matmuls start one
        # DMA-transfer earlier.
        wqk1_sb = {}
        xtq1_sb = {}
        for k in range(2):
            w = w_p.tile([P, 512], bf16, name=f"wqks{k}", tag=f"wqks{k}")
            nc.sync.dma_start(w, wqkv_d[P * k : P * (k + 1), 0:512])
            wqk1_sb[k] = w
            xh = x_p.tile([P, 512], bf16, name=f"xts{k}", tag=f"xts{k}")
            nc.scalar.dma_start(xh, xt_d[P * k : P * (k + 1), 0:512])
            xtq1_sb[k] = xh
        # pair-granular qk/x^T chunks keep the PE fed incrementally during
        # ramp-up; the V weights and later t-quarters go as single big DMAs
        # (HWDGE costs ~628ns per DMA instruction)
        wqk2_sb = [None]
        xtq0_sb = [None]
        for q in range(1, KD // 2):
            w = w_p.tile([P, 2 * 512], bf16, name=f"wqk{q}", tag=f"wqk{q}")
            nc.sync.dma_start(
                w.rearrange("p (two c) -> p two c", two=2),
                wqkv_d[2 * P * q : 2 * P * (q + 1), 0:512].rearrange(
                    "(two p) c -> p two c", two=2
                ),
            )
            wqk2_sb.append(w)
            xh = x_p.tile([P, 2 * 512], bf16, name=f"xt{q}_0",
                          tag=f"xt{q}_0")
            nc.scalar.dma_start(
                xh.rearrange("p (two t) -> p two t", two=2),
                xt_d[2 * P * q : 2 * P * (q + 1), 0:512].rearrange(
                    "(two p) t -> p two t", two=2
                ),
            )
            xtq0_sb.append(xh)
        wv_all = w_p.tile([P, KD * CD], bf16, name="wvall", tag="wvall")
        nc.sync.dma_start(
            wv_all.rearrange("p (k c) -> p k c", k=KD),
            wqkv_d[:, 512:768].rearrange("(k p) c -> p k c", k=KD),
        )
        xtq_sb = {}
        for tq in range(1, 4):
            xh = x_p.tile([P, KD * 512], bf16, name=f"xtq{tq}",
                          tag=f"xtq{tq}")
            (nc.scalar if tq < 3 else nc.sync).dma_start(
                xh.rearrange("p (k t) -> p k t", k=KD),
                xt_d[:, 512 * tq : 512 * (tq + 1)].rearrange(
                    "(k p) t -> p k t", k=KD
                ),
            )
            xtq_sb[tq] = xh

        def wqk_ap(k, lo, hi):
            if k < 2:
                return wqk1_sb[k][:, lo:hi]
            return wqk2_sb[k // 2][:, 512 * (k % 2) + lo : 512 * (k % 2) + hi]

        def wv_ap(k):
            return wv_all[:, CD * k : CD * (k + 1)]

        def xt_ap(k, lo, hi):
            tq, r = divmod(lo, 512)
            if tq == 0:
                if k < 2:
                    return xtq1_sb[k][:, r : r + (hi - lo)]
                return xtq0_sb[k // 2][
                    :, 512 * (k % 2) + r : 512 * (k % 2) + r + (hi - lo)
                ]
            return xtq_sb[tq][:, 512 * k + r : 512 * k + r + (hi - lo)]

        bqk = const_p.tile([P, 4], f32, name="bqk_sb")
        nc.gpsimd.dma_start(bqk, bqk_d)
        bv = const_p.tile([P, CD], bf16, name="bv_sb")
        nc.gpsimd.dma_start(bv, bv_d)
        mask = const_p.tile([P, P], bf16, name="mask_sb")
        nc.gpsimd.dma_start(mask, mask_d)
        maskn = const_p.tile([P, P], bf16, name="maskn_sb")
        nc.gpsimd.dma_start(maskn, maskn_d)
        onescol = const_p.tile([P, 2 * HPC], bf16, name="onescol_sb")
        nc.gpsimd.dma_start(onescol, onescol_d)
        ident = const_p.tile([P, P], bf16, name="ident_sb")
        nc.gpsimd.dma_start(ident, ident_d)
        wp_sb = []
        for c in range(CD // P):
            t = w_p.tile([P, D], bf16, name=f"wp{c}", tag=f"wp{c}")
            nc.gpsimd.dma_start(t, wp_d[P * c : P * (c + 1), :])
            wp_sb.append(t)

        # persistent activation buffers
        qkt_sb = {}
        for m in range(4):
            for j in range(TB):
                qkt_sb[(m, j)] = qkt_p.tile(
                    [P, 512], bf16, name=f"qkt{m}_{j}", tag=f"qkt{m}_{j}"
                )
        v_sb = []
        for i in range(TT):
            v_sb.append(
                v_p.tile([P, HPC * (H + 2)], bf16, name=f"v{i}", tag=f"v{i}")
            )
        # one tile per (t-block, head-pair, 128-col t-subtile): each proj
        # transpose then depends only on its own diagonal's normalize
        attn_t = {
            (tb, c, dj): attn_p.tile([P, P], bf16, name=f"attn{tb}_{c}_{dj}",
                                     tag=f"attn{tb}_{c}_{dj}")
            for tb in range(TB)
            for c in range(2)
            for dj in range(4)
        }
        G = H + 2

        def qk_block(j, pend=None):
            """qk^T for 512-wide t-block j (x^T comes from the host).
            Block 0 runs k-outer with all four m psums live (2 on the score
            banks, idle until a(0)) so the PE consumes each (wqk, x^T) DMA
            pair the moment it lands instead of stalling m-by-m.  `pend` is
            the previous attention block's deferred final AV: it is emitted
            after the first qk psum group, by which time its exp is done."""
            off = 512 * j
            if j == 0:
                pss = {
                    m: psp.tile([P, 512], f32, name="qkp",
                                tag=("wps" if m < 2 else "sp"))
                    for m in (0, 2, 1, 3)
                }
                for k in range(KD):
                    for m in (0, 2, 1, 3):
                        nc.tensor.matmul(
                            pss[m],
                            wqk_ap(k, P * m, P * (m + 1)),
                            xt_ap(k, off, off + 512),
                            start=(k == 0),
                            stop=(k == KD - 1),
                        )
                for m in (0, 2, 1, 3):
                    nc.vector.tensor_scalar_add(
                        qkt_sb[(m, j)], pss[m], bqk[:, m : m + 1]
                    )
            else:
                for m in (0, 2, 1, 3):  # head-pair 0 needs m0+m2 first
                    ps = psp.tile([P, 512], f32, name="qkp", tag="wps")
                    for k in range(KD):
                        nc.tensor.matmul(
                            ps,
                            wqk_ap(k, P * m, P * (m + 1)),
                            xt_ap(k, off, off + 512),
                            start=(k == 0),
                            stop=(k == KD - 1),
                        )
                    # psum->sbuf drain with the per-partition qk bias folded
                    nc.vector.tensor_scalar_add(
                        qkt_sb[(m, j)], ps, bqk[:, m : m + 1]
                    )
                    if pend is not None:
                        pend()
                        pend = None

        def v_block(j, pend=None):
            off = 512 * j
            for ti in range(4):
                g = 4 * j + ti
                ps = psp.tile([P, CD], f32, name="vp", tag="wps")
                for k in range(KD):
                    nc.tensor.matmul(
                        ps,
                        xt_ap(k, off + P * ti, off + P * (ti + 1)),
                        wv_ap(k),
                        start=(k == 0),
                        stop=(k == KD - 1),
                    )
                # psum->sbuf drain with the (partition-broadcast) V bias
                vg = v_sb[g].rearrange("p (g c) -> p g c", g=HPC)
                nc.vector.tensor_add(
                    vg[:, :, 0:H],
                    ps.rearrange("p (g c) -> p g c", g=HPC),
                    bv.rearrange("p (g c) -> p g c", g=HPC),
                )
                nc.gpsimd.tensor_copy(
                    vg[:, :, H : H + 2],
                    onescol.rearrange("p (g c) -> p g c", c=2),
                )
                if pend is not None:
                    pend()
                    pend = None

        def v_units(j):
            """v_block as filler chunks (one t-tile each): the last block's
            V tiles are only consumed by LATE AV s-tiles, so they can ride
            inside attention(3) instead of serializing before it."""
            off = 512 * j

            def unit(ti):
                def emit():
                    g = 4 * j + ti
                    ps = psp.tile([P, CD], f32, name="vp", tag="wps")
                    for k in range(KD):
                        nc.tensor.matmul(
                            ps,
                            xt_ap(k, off + P * ti, off + P * (ti + 1)),
                            wv_ap(k),
                            start=(k == 0),
                            stop=(k == KD - 1),
                        )
                    vg = v_sb[g].rearrange("p (g c) -> p g c", g=HPC)
                    nc.vector.tensor_add(
                        vg[:, :, 0:H],
                        ps.rearrange("p (g c) -> p g c", g=HPC),
                        bv.rearrange("p (g c) -> p g c", g=HPC),
                    )
                    nc.gpsimd.tensor_copy(
                        vg[:, :, H : H + 2],
                        onescol.rearrange("p (g c) -> p g c", c=2),
                    )
                return emit

            return [unit(ti) for ti in range(4)]

        def attention(tb, fillers=None, pend=None):
            """S^T -> exp -> AV for 512-wide t-block tb, heads processed in
            pairs (partition bases 0 and 64).  The AV matmuls of s-tile i are
            emitted AFTER the scores of s-tile i+1 (software pipeline): exp(i)
            then runs on ACT while the PE computes scores(i+1), so the
            in-order PE stream reaches AV(i) with its input already done.
            The final s-tile's AV is RETURNED as a deferred callback and
            emitted behind the next section's leading PE work, so the
            pipeline never flushes against a fresh exp.  After each s-tile,
            pops one emit-callback from `fillers` (ready projection work) to
            cover the remaining ACT-PE deficit."""
            for hp in range(2):
                h0 = 2 * hp
                mq, mk = hp, 2 + hp
                acc_t = [
                    psp.tile([P, 4 * 66], f32, name="accp", tag=f"accp{a}",
                             bufs=1)
                    for a in range(2)
                ]
                n_s = 4 * tb + 4  # s-tiles 0 .. 4*tb+3

                def emit_av(i, et, etd, acc_t=acc_t, hp=hp, h0=h0):
                    # per-hp state bound as defaults: the final AV of each
                    # head-pair is deferred and runs after `hp` has moved on
                    first = max(0, i - 4 * tb)
                    dj = i - 4 * tb
                    for jj in range(first, 4):
                        jglob = 4 * tb + jj
                        for hh in range(2):
                            if jj == dj and etd is not None:
                                lhs_e = etd[:, P * hh : P * (hh + 1)]
                            else:
                                lhs_e = et[
                                    :, 512 * hh + P * jj : 512 * hh + P * (jj + 1)
                                ]
                            # start=True clears has_written for the WHOLE
                            # psum bank: only the first group per bank
                            # issues it.
                            nc.tensor.matmul(
                                acc_t[hh][:, 66 * jj : 66 * jj + 66],
                                lhs_e,
                                v_sb[i][:, G * (h0 + hh) : G * (h0 + hh) + 66],
                                start=(i == 0 and jj == 0),
                                stop=(i == jglob),
                                skip_group_check=True,
                            )
                    if 0 <= dj <= 3:
                        # acc group dj just received its last (diagonal)
                        # contribution: normalize it now so the psum bank
                        # region drains while later s-tiles still accumulate
                        for hh in range(2):
                            s0 = 66 * dj
                            rec = small_p.tile([P, 1], f32, name="rec",
                                               tag="rec")
                            nc.vector.reciprocal(
                                rec, acc_t[hh][:, s0 + H : s0 + H + 1]
                            )
                            nc.vector.tensor_scalar_mul(
                                attn_t[(tb, hp, dj)][:, H * hh : H * (hh + 1)],
                                acc_t[hh][:, s0 : s0 + H],
                                rec,
                            )
                    if fillers and dj < 0:
                        # diagonal s-tiles have small exps and little PE
                        # deficit: save fillers for the full-width periods
                        fillers.pop(0)()

                for i in range(n_s):
                    first = max(0, i - 4 * tb)  # first valid jj in block
                    c0 = P * first
                    dj_ = i - 4 * tb
                    # the final block masks its diagonal IN PSUM (an extra
                    # -200-additive matmul): its AVs then read et directly,
                    # keeping the DVE mask-mul off the endgame critical path
                    inpsum_mask = tb == 3 and 0 <= dj_ <= 3
                    sps = psp.tile([P, 1024], f32, name="sp", tag="sp",
                                   bufs=2)
                    for hh, pb in ((0, 0), (1, 64)):
                        if inpsum_mask:
                            nc.tensor.matmul(
                                sps[:, 512 * hh + c0 : 512 * hh + c0 + P],
                                qkt_sb[(mk, i // 4)][
                                    pb : pb + H, P * (i % 4) : P * (i % 4 + 1)
                                ],
                                qkt_sb[(mq, tb)][pb : pb + H, c0 : c0 + P],
                                start=True,
                                stop=False,
                            )
                            nc.tensor.matmul(
                                sps[:, 512 * hh + c0 : 512 * hh + c0 + P],
                                ident,
                                maskn,
                                start=False,
                                stop=True,
                            )
                            if c0 + P < 512:
                                nc.tensor.matmul(
                                    sps[:, 512 * hh + c0 + P : 512 * hh + 512],
                                    qkt_sb[(mk, i // 4)][
                                        pb : pb + H,
                                        P * (i % 4) : P * (i % 4 + 1)
                                    ],
                                    qkt_sb[(mq, tb)][pb : pb + H, c0 + P : 512],
                                    start=True,
                                    stop=True,
                                )
                        else:
                            nc.tensor.matmul(
                                sps[:, 512 * hh + c0 : 512 * hh + 512],
                                qkt_sb[(mk, i // 4)][
                                    pb : pb + H, P * (i % 4) : P * (i % 4 + 1)
                                ],
                                qkt_sb[(mq, tb)][pb : pb + H, c0:512],
                                start=True,
                                stop=True,
                            )
                    et = e_p.tile([P, 1024], bf16, name="et", tag="et")
                    if first:
                        nc.scalar.activation(
                            et.rearrange("p (g c) -> p g c", g=2)[
                                :, :, c0:512
                            ],
                            sps.rearrange("p (g c) -> p g c", g=2)[
                                :, :, c0:512
                            ],
                            mybir.ActivationFunctionType.Exp,
                            scale=1.0 / math.sqrt(H),
                        )
                    else:
                        nc.scalar.activation(
                            et,
                            sps,
                            mybir.ActivationFunctionType.Exp,
                            scale=1.0 / math.sqrt(H),
                        )
                    dj = i - 4 * tb  # diagonal jj of this s-tile, if any
                    etd = None
                    if 0 <= dj <= 3 and not inpsum_mask:
                        # masked diagonal sub-tiles go to a separate tile so
                        # the non-diagonal AV matmuls don't serialize behind
                        # the mask write (tile-granular dependency tracking)
                        etd = e_p.tile([P, 2 * P], bf16, name="etd", tag="etd",
                                       bufs=2)
                        for hh in range(2):
                            nc.vector.tensor_mul(
                                etd[:, P * hh : P * (hh + 1)],
                                et[:, 512 * hh + P * dj : 512 * hh + P * (dj + 1)],
                                mask,
                            )
                    if pend is not None:
                        pend()
                    pend = (
                        lambda i=i, et=et, etd=etd, f=emit_av: f(i, et, etd)
                    )
            return pend

        def proj_transpose_unit(jb, c, attnT):
            """attn^T for one 128-col group of t-block jb."""
            def emit():
                pt = psp.tile([P, 512], bf16, name="atp", tag="wps")
                for ti in range(4):
                    nc.tensor.transpose(
                        pt[:, P * ti : P * (ti + 1)],
                        attn_t[(jb, c, ti)],
                        ident,
                    )
                at = e_p.tile([P, 512], bf16, name="at", tag="at", bufs=4)
                nc.vector.tensor_copy(at, pt)
                attnT[c] = at
            return emit

        def proj_mm_unit(jb, jl, n, attnT, last=False):
            """One 512-wide half of y = attn @ wp for one 128-row t-tile.
            The final block drains on both DVE and ACT so its post-attention
            tail chain is as short as possible."""
            def emit():
                jt = 4 * jb + jl
                # the final block's projection runs after all attention:
                # the score psum slots are free then
                ps = psp.tile([P, 512], f32, name="yp",
                              tag=(("sp" if (2 * jl + n) % 2 == 0 else "wps")
                                   if last else "wps"))
                for c in range(CD // P):
                    nc.tensor.matmul(
                        ps,
                        attnT[c][:, P * jl : P * (jl + 1)],
                        wp_sb[c][:, 512 * n : 512 * (n + 1)],
                        start=(c == 0),
                        stop=(c == CD // P - 1),
                    )
                if last and jl < 2:
                    # both halves drain (DVE || ACT) into one tile: a single
                    # HWDGE descriptor per t-tile keeps the tail off the
                    # serialized HWDGE queue
                    dve_first = (jl % 2 == 0)
                    if n == 0:
                        ylast[jt] = small_p.tile([P, D], bf16, name="ysbL",
                                                 tag="ysbL", bufs=4)
                        (nc.vector.tensor_copy if dve_first
                         else nc.scalar.copy)(ylast[jt][:, 0:512], ps)
                    else:
                        (nc.scalar.copy if dve_first
                         else nc.vector.tensor_copy)(
                            ylast[jt][:, 512:1024], ps)
                        # odd t-tiles go out via SWDGE: parallel to the
                        # serialized HWDGE descriptor stage at the tail
                        (nc.sync if jl % 2 == 0 else nc.gpsimd).dma_start(
                            y_d[P * jt : P * (jt + 1), :], ylast[jt]
                        )
                elif last:
                    # final two t-tiles: per-half drains on alternating
                    # engines with their own immediate DMAs - the shortest
                    # end-of-kernel chain
                    ysb = small_p.tile([P, 512], bf16, name="ysbF",
                                       tag="ysbF", bufs=4)
                    ((nc.vector.tensor_copy, nc.scalar.copy)[
                        (2 * jl + n) % 2])(ysb, ps)
                    (nc.sync, nc.gpsimd, nc.scalar, nc.sync)[
                        2 * (jl - 2) + n].dma_start(
                        y_d[P * jt : P * (jt + 1), 512 * n : 512 * (n + 1)],
                        ysb,
                    )
                else:
                    ysb = small_p.tile([P, 512], bf16, name="ysb", tag="ysb",
                                       bufs=8)
                    nc.vector.tensor_copy(ysb, ps)
                    (nc.sync if jl % 2 == 0 else nc.scalar).dma_start(
                        y_d[P * jt : P * (jt + 1), 512 * n : 512 * (n + 1)],
                        ysb,
                    )
            return emit

        ylast = {}

        def projection_units(jb, last=False):
            attnT = {}
            units = [proj_transpose_unit(jb, c, attnT) for c in range(2)]
            units += [
                proj_mm_unit(jb, jl, n, attnT, last)
                for jl in range(4)
                for n in range(2)
            ]
            return units, attnT

        # emission order: chunk-k attention (ACT-bound) overlaps phase 1 of
        # chunk k+1 (PE-bound); the last attention block's stalls are filled
        # with the earlier blocks' (ready) projection work.  The final
        # block's first attn^T group goes in late (its head-pair-0 rows are
        # done once a(3) hp=0 finishes) to shorten the post-attention tail.
        qk_block(0)
        v_block(0)
        pend = attention(0)
        qk_block(1, pend)
        fv1 = v_units(1)
        pend = attention(1, fillers=fv1)
        for f in fv1:
            f()
        qk_block(2, pend)
        fv2 = v_units(2)
        pend = attention(2, fillers=fv2)
        for f in fv2:
            f()
        qk_block(3, pend)
        v_block(3)
        u0, _ = projection_units(0)
        u1, _ = projection_units(1)
        u2, _ = projection_units(2)
        u3, attnT3 = projection_units(3, last=True)
        fillers = u0 + u1 + u2 + [u3[0]]
        pend = attention(3, fillers=fillers)
        for f in fillers:
            f()
        pend()
        for f in u3[1:]:
            f()

    nc.compile()
    return nc


def _get_module(mm_dt_name: str):
    if mm_dt_name not in _CACHE:
        _CACHE[mm_dt_name] = _build_module(mm_dt_name)
    return _CACHE[mm_dt_name]


def kernel(x, w_attn, b_attn, w_proj, b_proj, mm_dt_name: str = "float32r",
           trace: bool = False):
    from concourse.bass_utils import run_bass_kernel_spmd

    x = np.asarray(x, dtype=np.float32)
    w_attn = np.asarray(w_attn, dtype=np.float32)
    b_attn = np.asarray(b_attn, dtype=np.float32)
    w_proj = np.asarray(w_proj, dtype=np.float32)
    b_proj = np.asarray(b_proj, dtype=np.float32)

    nc = _get_module(mm_dt_name)

    import ml_dtypes

    bf = np.dtype(ml_dtypes.bfloat16)
    ident = np.eye(P, dtype=bf)
    mask = np.triu(np.ones((P, P), dtype=bf))
    maskn = np.where(
        np.tril(np.ones((P, P), dtype=bool), -1), -200.0, 0.0
    ).astype(bf)

    in_maps = []
    for core in range(N_CORES):
        b = core // 4
        g = core % 4
        c0 = CD * g
        wq = w_attn[:, c0 : c0 + CD]
        wk = w_attn[:, D + c0 : D + c0 + CD]
        wv = w_attn[:, 2 * D + c0 : 2 * D + c0 + CD]
        bq = b_attn[c0 : c0 + CD]
        bk = b_attn[D + c0 : D + c0 + CD]
        bvv = b_attn[2 * D + c0 : 2 * D + c0 + CD]
        in_maps.append(
            {
                "xt": np.ascontiguousarray(x[b].T).astype(bf),
                "wqkv": np.ascontiguousarray(
                    np.concatenate([wq, wk, wv], axis=1)
                ).astype(bf),
                "bqk": np.concatenate([bq, bk]).reshape(4, P).T.copy(),
                "bv": np.broadcast_to(bvv[None, :], (P, CD)).astype(bf).copy(),
                "wp": np.ascontiguousarray(w_proj[c0 : c0 + CD, :]).astype(bf),
                "ident": ident,
                "mask": mask,
                "maskn": maskn,
                "onescol": np.tile(np.array([1.0, 0.0], bf), (P, HPC)),
            }
        )

    res = run_bass_kernel_spmd(
        nc, in_maps, core_ids=list(range(N_CORES)), trace=trace
    )

    out = np.zeros((B, T, D), dtype=np.float32)
    for core in range(N_CORES):
        out[core // 4] += res.results[core]["y"]
    out += b_proj[None, None, :]
    if trace:
        kernel.last_result = res
    return out
